# revision 10
# baseline (speedup 1.0000x reference)
"""Trainium2 Bass kernel for nn_AetheriumTransformerBlock (8-core SPMD).

Strategy:
  - Sequence-parallel attention: each core owns 128 of the 1024 tokens;
    K/V computed locally then AllGathered (bf16).
  - On-device top-2 routing (router matmul in fp32 so expert selection is
    bit-stable vs the fp32 reference), replicated on all cores.
  - Capacity-based token compaction via triangular-matmul prefix sums +
    indirect DMA scatter/gather.
  - Experts inter-dim sharded: every core computes a 1/8 slice of every
    expert's intermediate dim on that expert's compacted tokens (bf16).
    The creativity tanh branch lands in per-core columns via a placement
    matmul whose 0.2-scaled one-hot matrix is a per-core input.
  - Per-token combine by gathering each token's two expert rows, then a
    ReduceScatter(add) returns the summed MoE output to the owning core.

Self-contained: hardcodes all shapes; host-side prep only reorders/casts
weights and slices inputs.
"""
import numpy as np
import ml_dtypes

import concourse.bass as bass
import concourse.bacc as bacc
import concourse.mybir as mybir
from concourse.tile import TileContext
from concourse.bass_utils import run_bass_kernel_spmd
from concourse.masks import make_identity, make_upper_triangular

F32 = mybir.dt.float32
BF16 = mybir.dt.bfloat16
I32 = mybir.dt.int32
AF = mybir.ActivationFunctionType
ALU = mybir.AluOpType
AX = mybir.AxisListType

P = 128
T = 1024
H = 1024
NH = 4
HD = 256
LAT = 256
E = 8
NCORE = 8
CAP = 384                 # per-expert token capacity (3 chunks of 128)
NSLOT = E * CAP           # 3072
TRASH = NSLOT             # overflow slot row
KINDS = ('quantum', 'quantum', 'creativity', 'creativity',
         'general', 'general', 'general', 'general')
INTER = {k: (4096 if k == 'quantum' else 2048) for k in KINDS}
SCALE = HD ** -0.5
EPS = 1e-5
HC = H // P               # 8 hidden chunks
TC = T // P               # 8 token chunks
SC = CAP // P             # 3 slot chunks per expert
RG = [list(range(NCORE))]


def build_nc(reps=1):
    nc = bacc.Bacc("TRN2", target_bir_lowering=False, debug=False,
                   num_devices=NCORE)
    inp = {}

    def din(name, shape, dtype):
        inp[name] = nc.dram_tensor(name, shape, dtype, kind="ExternalInput")

    din("x_own", [P, H], F32)
    din("cos_own", [P, HD // 2], F32)
    din("sin_own", [P, HD // 2], F32)
    din("qw_t", [H, H], BF16)
    din("kw_t", [H, LAT], BF16)
    din("vw_t", [H, LAT], BF16)
    din("ow_t", [H, H], BF16)
    din("rw_t", [H, E], F32)
    din("cre_sel", [P, H], BF16)          # 0.2-scaled one-hot placement
    for e, kind in enumerate(KINDS):
        ic = INTER[kind] // NCORE
        din(f"wg{e}", [H, ic], BF16)
        din(f"wu{e}", [H, ic], BF16)
        din(f"wd{e}", [ic, H], BF16)
        if kind == 'creativity':
            din(f"wc{e}", [H, P], BF16)

    y_out = nc.dram_tensor("y_out", [P, H], F32, kind="ExternalOutput")

    scr = {}
    scr["kv_in"] = nc.dram_tensor("kv_in", [P, 512], BF16)
    scr["kv_all"] = nc.dram_tensor("kv_all", [T, 512], BF16, addr_space="Shared")
    scr["h2_in"] = nc.dram_tensor("h2_in", [P, H], BF16)
    scr["h2_all"] = nc.dram_tensor("h2_all", [T, H], BF16, addr_space="Shared")
    scr["lg_in"] = nc.dram_tensor("lg_in", [P, E], F32)
    scr["lg_all"] = nc.dram_tensor("lg_all", [T, E], F32, addr_space="Shared")
    scr["tid_w"] = nc.dram_tensor("tid_w", [NSLOT + P, 2], F32)
    scr["ycomp"] = nc.dram_tensor("ycomp", [NSLOT + P, H], BF16)
    scr["rs_in"] = nc.dram_tensor("rs_in", [T, H], F32)
    scr["rs_out"] = nc.dram_tensor("rs_out", [P, H], F32)

    with TileContext(nc) as tc:
        for _rep in range(reps):
            build_program(nc, tc, inp, y_out, scr)
    nc.compile()
    return nc


def build_program(nc, tc, inp, y_out, scr):
    import contextlib
    stack = contextlib.ExitStack()
    with stack:
        const = stack.enter_context(tc.tile_pool(name="const", bufs=1))
        sb = stack.enter_context(tc.tile_pool(name="sb", bufs=2))
        wp = stack.enter_context(tc.tile_pool(name="wp", bufs=2))
        ps = stack.enter_context(tc.tile_pool(name="ps", bufs=2, space="PSUM"))
        pst = stack.enter_context(tc.tile_pool(name="pst", bufs=2, space="PSUM"))

        # ---- constants ----
        idn_b = const.tile([P, P], BF16)
        make_identity(nc, idn_b[:])
        idn_f = const.tile([P, P], F32)
        make_identity(nc, idn_f[:])
        ones_b = const.tile([P, P], BF16)
        nc.gpsimd.memset(ones_b[:], 1.0)
        triu_b = const.tile([P, P], BF16)
        make_upper_triangular(nc, triu_b[:], val=1.0, diag=False)
        base_iota = const.tile([P, E], I32)
        nc.gpsimd.iota(base_iota[:], pattern=[[CAP, E]], base=0,
                       channel_multiplier=0)
        base_f = const.tile([P, E], F32)
        nc.vector.tensor_copy(base_f[:], base_iota[:])
        cos_t = const.tile([P, HD // 2], F32)
        nc.sync.dma_start(cos_t[:], inp["cos_own"][:])
        sin_t = const.tile([P, HD // 2], F32)
        nc.sync.dma_start(sin_t[:], inp["sin_own"][:])
        zero_f = const.tile([P, 64], F32)
        nc.gpsimd.memset(zero_f[:], 0.0)
        eps_t = const.tile([P, 1], F32)
        nc.gpsimd.memset(eps_t[:], EPS)
        zero_b = const.tile([P, H], BF16)
        nc.gpsimd.memset(zero_b[:], 0.0)
        cre_sel = const.tile([P, H], BF16)
        nc.sync.dma_start(cre_sel[:], inp["cre_sel"][:])

        x_t = const.tile([P, H], F32)
        nc.sync.dma_start(x_t[:], inp["x_own"][:])

        # zero the scratch padding regions
        nperp = 2 * (NSLOT + P) // P        # 50 f32 per partition
        nc.sync.dma_start(
            scr["tid_w"][:].rearrange("(a p) b -> p a b", p=P),
            zero_f[:, :nperp].rearrange("p (a b) -> p a b", b=2))
        nc.sync.dma_start(scr["ycomp"][NSLOT:NSLOT + P, :], zero_b[:])

        def ln_pair(src_f32, out_bf, out_f32=None):
            mu = sb.tile([P, 1], F32, tag="ln_small")
            nc.vector.tensor_reduce(out=mu[:], in_=src_f32, axis=AX.X,
                                    op=ALU.add, negate=True)
            negmu = sb.tile([P, 1], F32, tag="ln_small")
            nc.vector.tensor_scalar_mul(negmu[:], mu[:], 1.0 / H)
            cent = sb.tile([P, H], F32, tag="ln_big")
            nc.vector.tensor_scalar_add(cent[:], src_f32, negmu[:, :1])
            varsum = sb.tile([P, 1], F32, tag="ln_small")
            sq = sb.tile([P, H], F32, tag="ln_big")
            nc.scalar.activation(out=sq[:], in_=cent[:], func=AF.Square,
                                 accum_out=varsum[:, :1])
            std = sb.tile([P, 1], F32, tag="ln_small")
            nc.scalar.activation(out=std[:], in_=varsum[:], func=AF.Sqrt,
                                 scale=1.0 / H, bias=eps_t[:, :1])
            rstd = sb.tile([P, 1], F32, tag="ln_small")
            nc.vector.reciprocal(rstd[:], std[:])
            nc.vector.tensor_scalar_mul(out_bf, cent[:], rstd[:, :1])
            if out_f32 is not None:
                nc.vector.tensor_scalar_mul(out_f32, cent[:], rstd[:, :1])

        def transpose_to(dst_ap, src_ap, dtype):
            pt = pst.tile([P, P], dtype, tag="tp")
            n_in, f_in = src_ap.shape
            idn = idn_b if dtype == BF16 else idn_f
            nc.tensor.transpose(out=pt[:f_in, :n_in], in_=src_ap,
                                identity=idn[:n_in, :n_in])
            nc.vector.tensor_copy(dst_ap, pt[:f_in, :n_in])

        def load_w(name, rows, cols, tag, bufs=2, dtype=BF16):
            """Load DRAM [rows, cols] as SBUF [128, (rows/128)*cols]."""
            cc = rows // P
            wt = wp.tile([P, cc * cols], dtype, tag=tag, bufs=bufs)
            nc.sync.dma_start(
                wt[:].rearrange("p (c n) -> p c n", c=cc),
                inp[name][:].rearrange("(c p) n -> p c n", p=P))
            return wt

        # ================= attention =================
        h_b = sb.tile([P, H], BF16, tag="h_b", bufs=1)
        ln_pair(x_t[:], h_b[:])
        hT = sb.tile([P, HC * P], BF16, tag="hT", bufs=1)
        for kc in range(HC):
            transpose_to(hT[:, kc * P:(kc + 1) * P],
                         h_b[:, kc * P:(kc + 1) * P], BF16)

        def proj(w_name, ncols, psum_tag):
            """[128 tok, ncols] = h @ W.T with monolithic weight load."""
            wt = load_w(w_name, H, ncols, tag="w_att", bufs=2)
            nsl = [(n0, min(512, ncols - n0)) for n0 in range(0, ncols, 512)]
            pts = [ps.tile([P, 512], F32, tag=psum_tag, name=f'pj{i}') for i in range(len(nsl))]
            for kc in range(HC):
                for (n0, nn), pt in zip(nsl, pts):
                    nc.tensor.matmul(
                        pt[:, :nn],
                        lhsT=hT[:, kc * P:(kc + 1) * P],
                        rhs=wt[:, kc * ncols + n0:kc * ncols + n0 + nn],
                        start=(kc == 0), stop=(kc == HC - 1))
            return [(pt, nn) for (n0, nn), pt in zip(nsl, pts)]

        def rope_from_psum(dst_bf, psums, ncols):
            for h0 in range(0, ncols, HD):
                pt = psums[h0 // 512][0]
                off = h0 % 512
                x1 = pt[:, off:off + HD:2]
                x2 = pt[:, off + 1:off + HD:2]
                t1 = sb.tile([P, HD // 2], F32, tag="rope")
                nc.vector.tensor_tensor(out=t1[:], in0=x1, in1=cos_t[:],
                                        op=ALU.mult)
                t2 = sb.tile([P, HD // 2], F32, tag="rope")
                nc.vector.tensor_tensor(out=t2[:], in0=x2, in1=sin_t[:],
                                        op=ALU.mult)
                nc.vector.tensor_tensor(out=dst_bf[:, h0:h0 + HD:2],
                                        in0=t1[:], in1=t2[:], op=ALU.subtract)
                nc.vector.tensor_tensor(out=t1[:], in0=x2, in1=cos_t[:],
                                        op=ALU.mult)
                nc.vector.tensor_tensor(out=t2[:], in0=x1, in1=sin_t[:],
                                        op=ALU.mult)
                nc.vector.tensor_tensor(out=dst_bf[:, h0 + 1:h0 + HD:2],
                                        in0=t1[:], in1=t2[:], op=ALU.add)

        with tc.tile_pool(name="ps_sc", bufs=2, space="PSUM") as ps_sc:
            kv_sb = sb.tile([P, 512], BF16, tag="kv", bufs=1)
            k_ps = proj("kw_t", LAT, "acc")
            rope_from_psum(kv_sb[:, 0:LAT], k_ps, LAT)
            v_ps = proj("vw_t", LAT, "acc")
            nc.vector.tensor_copy(kv_sb[:, LAT:512], v_ps[0][0][:, :LAT])
            nc.sync.dma_start(scr["kv_in"][:], kv_sb[:])
            nc.gpsimd.collective_compute(
                "AllGather", ALU.bypass, replica_groups=RG,
                ins=[scr["kv_in"][:]], outs=[scr["kv_all"][:]])

            q_b = sb.tile([P, H], BF16, tag="q_b", bufs=1)
            q_ps = proj("qw_t", H, "acc")
            rope_from_psum(q_b[:], q_ps, H)
            qT = sb.tile([P, HC * P], BF16, tag="qT", bufs=1)
            for kc in range(HC):
                transpose_to(qT[:, kc * P:(kc + 1) * P],
                             q_b[:, kc * P:(kc + 1) * P], BF16)

            kT = sb.tile([P, 2 * T], BF16, tag="kT", bufs=1)
            for dc in range(2):
                nc.sync.dma_start(kT[:, dc * T:(dc + 1) * T],
                                  scr["kv_all"][:, dc * P:(dc + 1) * P],
                                  transpose=True)
            v_all = sb.tile([P, TC * LAT], BF16, tag="v_all", bufs=1)
            nc.sync.dma_start(
                v_all[:].rearrange("p (c d) -> p c d", c=TC),
                scr["kv_all"][:, LAT:512].rearrange("(c p) d -> p c d", p=P))

            ao_b = sb.tile([P, H], BF16, tag="ao_b", bufs=1)
            for hh in range(NH):
                sc_ps = ps_sc.tile([P, T], F32, tag="scores")
                for n0 in (0, 512):
                    for dc in range(2):
                        nc.tensor.matmul(
                            sc_ps[:, n0:n0 + 512],
                            lhsT=qT[:, (hh * 2 + dc) * P:(hh * 2 + dc + 1) * P],
                            rhs=kT[:, dc * T + n0:dc * T + n0 + 512],
                            start=(dc == 0), stop=(dc == 1))
                negm = sb.tile([P, 1], F32, tag="sm_small")
                nc.vector.tensor_reduce(out=negm[:], in_=sc_ps[:], axis=AX.X,
                                        op=ALU.max, negate=True)
                negm_s = sb.tile([P, 1], F32, tag="sm_small")
                nc.vector.tensor_scalar_mul(negm_s[:], negm[:], SCALE)
                attn = sb.tile([P, T], BF16, tag="attn")
                sume = sb.tile([P, 1], F32, tag="sm_small")
                nc.scalar.activation(out=attn[:], in_=sc_ps[:], func=AF.Exp,
                                     scale=SCALE, bias=negm_s[:, :1],
                                     accum_out=sume[:, :1])
                rcp = sb.tile([P, 1], F32, tag="sm_small")
                nc.vector.reciprocal(rcp[:], sume[:])
                attnT = sb.tile([P, T], BF16, tag="attnT")
                for kc in range(TC):
                    transpose_to(attnT[:, kc * P:(kc + 1) * P],
                                 attn[:, kc * P:(kc + 1) * P], BF16)
                av_ps = ps.tile([P, LAT], F32, tag="acc")
                for kc in range(TC):
                    nc.tensor.matmul(
                        av_ps[:],
                        lhsT=attnT[:, kc * P:(kc + 1) * P],
                        rhs=v_all[:, kc * LAT:(kc + 1) * LAT],
                        start=(kc == 0), stop=(kc == TC - 1))
                nc.vector.tensor_scalar_mul(ao_b[:, hh * HD:(hh + 1) * HD],
                                            av_ps[:], rcp[:, :1])

            aoT = sb.tile([P, HC * P], BF16, tag="aoT", bufs=1)
            for kc in range(HC):
                transpose_to(aoT[:, kc * P:(kc + 1) * P],
                             ao_b[:, kc * P:(kc + 1) * P], BF16)
            x2_t = const.tile([P, H], F32)
            ow_sb = load_w("ow_t", H, H, tag="w_att", bufs=2)
            o_pts = [ps.tile([P, 512], F32, tag="acc", name=f'op{i}') for i in range(2)]
            for kc in range(HC):
                for i, n0 in enumerate((0, 512)):
                    nc.tensor.matmul(
                        o_pts[i][:],
                        lhsT=aoT[:, kc * P:(kc + 1) * P],
                        rhs=ow_sb[:, kc * H + n0:kc * H + n0 + 512],
                        start=(kc == 0), stop=(kc == HC - 1))
            for i, n0 in enumerate((0, 512)):
                nc.vector.tensor_tensor(out=x2_t[:, n0:n0 + 512],
                                        in0=x_t[:, n0:n0 + 512],
                                        in1=o_pts[i][:], op=ALU.add)

        # ================= ln2 + router =================
        h2_f = const.tile([P, H], F32)
        h2_b = sb.tile([P, H], BF16, tag="h2_b", bufs=1)
        ln_pair(x2_t[:], h2_b[:], out_f32=h2_f[:])
        nc.sync.dma_start(scr["h2_in"][:], h2_b[:])
        nc.gpsimd.collective_compute(
            "AllGather", ALU.bypass, replica_groups=RG,
            ins=[scr["h2_in"][:]], outs=[scr["h2_all"][:]])

        h2T = sb.tile([P, HC * P], F32, tag="h2T", bufs=1)
        for kc in range(HC):
            transpose_to(h2T[:, kc * P:(kc + 1) * P],
                         h2_f[:, kc * P:(kc + 1) * P], F32)
        rw_sb = load_w("rw_t", H, E, tag="w_rt", bufs=1, dtype=F32)
        lg_ps = ps.tile([P, E], F32, tag="acc")
        for kc in range(HC):
            nc.tensor.matmul(lg_ps[:],
                             lhsT=h2T[:, kc * P:(kc + 1) * P],
                             rhs=rw_sb[:, kc * E:(kc + 1) * E],
                             start=(kc == 0), stop=(kc == HC - 1))
        lg_sb = sb.tile([P, E], F32, tag="lg_own", bufs=1)
        nc.vector.tensor_copy(lg_sb[:], lg_ps[:])
        nc.sync.dma_start(scr["lg_in"][:], lg_sb[:])
        nc.gpsimd.collective_compute(
            "AllGather", ALU.bypass, replica_groups=RG,
            ins=[scr["lg_in"][:]], outs=[scr["lg_all"][:]])

        # ================= routing (replicated, all tokens) =================
        msel_b = sb.tile([P, TC * E], BF16, tag="msel", bufs=1)
        wsel_all = sb.tile([P, TC * E], F32, tag="wsel", bufs=1)
        r1_all = sb.tile([P, TC * E], F32, tag="r1", bufs=1)
        for tch in range(TC):
            lg = sb.tile([P, E], F32, tag="lg_chunk")
            nc.sync.dma_start(lg[:], scr["lg_all"][tch * P:(tch + 1) * P, :])
            mx = sb.tile([P, 8], F32, tag="mx")
            nc.vector.max(out=mx[:], in_=lg[:])
            negm1 = sb.tile([P, 1], F32, tag="rt_small")
            nc.vector.tensor_scalar_mul(negm1[:], mx[:, 0:1], -1.0)
            e_all = sb.tile([P, E], F32, tag="rt_e")
            nc.scalar.activation(out=e_all[:], in_=lg[:], func=AF.Exp,
                                 bias=negm1[:, :1])
            d21 = sb.tile([P, 1], F32, tag="rt_small")
            nc.vector.tensor_tensor(out=d21[:], in0=mx[:, 1:2], in1=mx[:, 0:1],
                                    op=ALU.subtract)
            ed = sb.tile([P, 1], F32, tag="rt_small")
            nc.scalar.activation(out=ed[:], in_=d21[:], func=AF.Exp)
            denom = sb.tile([P, 1], F32, tag="rt_small")
            nc.vector.tensor_scalar_add(denom[:], ed[:], 1.0)
            rr = sb.tile([P, 1], F32, tag="rt_small")
            nc.vector.reciprocal(rr[:], denom[:])
            sel = sb.tile([P, E], F32, tag="rt_sel")
            nc.vector.tensor_tensor(out=sel[:], in0=lg[:],
                                    in1=mx[:, 1:2].to_broadcast([P, E]),
                                    op=ALU.is_ge)
            wsel = wsel_all[:, tch * E:(tch + 1) * E]
            nc.vector.tensor_scalar_mul(wsel, e_all[:], rr[:, :1])
            nc.vector.tensor_tensor(out=wsel, in0=wsel, in1=sel[:],
                                    op=ALU.mult)
            nc.vector.tensor_tensor(out=r1_all[:, tch * E:(tch + 1) * E],
                                    in0=lg[:],
                                    in1=mx[:, 0:1].to_broadcast([P, E]),
                                    op=ALU.is_ge)
            nc.vector.tensor_copy(msel_b[:, tch * E:(tch + 1) * E], sel[:])

        # ---- prefix-sum compaction + scatter ----
        slot12 = sb.tile([P, TC * 2], I32, tag="slot12", bufs=1)
        for tch in range(TC):
            pos_ps = ps.tile([P, E], F32, tag="acc")
            for kc in range(tch + 1):
                lhs = triu_b if kc == tch else ones_b
                nc.tensor.matmul(pos_ps[:],
                                 lhsT=lhs[:],
                                 rhs=msel_b[:, kc * E:(kc + 1) * E],
                                 start=(kc == 0), stop=(kc == tch))
            slotf = sb.tile([P, E], F32, tag="slotf")
            nc.vector.tensor_tensor(out=slotf[:], in0=pos_ps[:], in1=base_f[:],
                                    op=ALU.add)
            okc = sb.tile([P, E], F32, tag="okc")
            nc.vector.tensor_scalar(okc[:], pos_ps[:], float(CAP) - 0.5, None,
                                    ALU.is_le)
            sel = sb.tile([P, E], F32, tag="selw")
            nc.vector.tensor_scalar(sel[:], wsel_all[:, tch * E:(tch + 1) * E],
                                    0.0, None, ALU.is_gt)
            nc.vector.tensor_tensor(out=okc[:], in0=okc[:], in1=sel[:],
                                    op=ALU.mult)
            slot_ok = sb.tile([P, E], F32, tag="slot_ok")
            nc.vector.tensor_scalar_add(slot_ok[:], slotf[:], float(-TRASH))
            nc.vector.tensor_tensor(out=slot_ok[:], in0=slot_ok[:], in1=okc[:],
                                    op=ALU.mult)
            nc.vector.tensor_scalar_add(slot_ok[:], slot_ok[:], float(TRASH))
            r1 = r1_all[:, tch * E:(tch + 1) * E]
            r2 = sb.tile([P, E], F32, tag="r2")
            nc.vector.tensor_tensor(out=r2[:], in0=sel[:], in1=r1,
                                    op=ALU.subtract)
            tmp = sb.tile([P, E], F32, tag="rt_tmp")
            pair = sb.tile([P, 4], F32, tag="pair")
            tid_i = sb.tile([P, 1], I32, tag="tid_i")
            nc.gpsimd.iota(tid_i[:], pattern=[[0, 1]], base=tch * P,
                           channel_multiplier=1)
            nc.vector.tensor_copy(pair[:, 0:1], tid_i[:])
            nc.vector.tensor_copy(pair[:, 2:3], tid_i[:])
            nc.vector.tensor_tensor(out=tmp[:], in0=r1,
                                    in1=wsel_all[:, tch * E:(tch + 1) * E],
                                    op=ALU.mult)
            nc.vector.tensor_reduce(out=pair[:, 1:2], in_=tmp[:], axis=AX.X,
                                    op=ALU.add)
            nc.vector.tensor_tensor(out=tmp[:], in0=r2[:],
                                    in1=wsel_all[:, tch * E:(tch + 1) * E],
                                    op=ALU.mult)
            nc.vector.tensor_reduce(out=pair[:, 3:4], in_=tmp[:], axis=AX.X,
                                    op=ALU.add)
            sl_f = sb.tile([P, 2], F32, tag="sl_f")
            nc.vector.tensor_tensor(out=tmp[:], in0=r1, in1=slot_ok[:],
                                    op=ALU.mult)
            nc.vector.tensor_reduce(out=sl_f[:, 0:1], in_=tmp[:], axis=AX.X,
                                    op=ALU.add)
            nc.vector.tensor_tensor(out=tmp[:], in0=r2[:], in1=slot_ok[:],
                                    op=ALU.mult)
            nc.vector.tensor_reduce(out=sl_f[:, 1:2], in_=tmp[:], axis=AX.X,
                                    op=ALU.add)
            sl_i = slot12[:, tch * 2:(tch + 1) * 2]
            nc.vector.tensor_copy(sl_i, sl_f[:])
            nc.gpsimd.indirect_dma_start(
                out=scr["tid_w"][:],
                out_offset=bass.IndirectOffsetOnAxis(ap=sl_i[:, 0:1], axis=0),
                in_=pair[:, 0:2],
                in_offset=None)
            nc.gpsimd.indirect_dma_start(
                out=scr["tid_w"][:],
                out_offset=bass.IndirectOffsetOnAxis(ap=sl_i[:, 1:2], axis=0),
                in_=pair[:, 2:4],
                in_offset=None)

        # ================= experts =================
        with tc.tile_pool(name="ps_y", bufs=2, space="PSUM") as ps_y:
            for e, kind in enumerate(KINDS):
                ic = INTER[kind] // NCORE
                mi_n = ic // P
                xT = sb.tile([P, HC * CAP], BF16, tag="xT")
                wsl = sb.tile([P, SC], F32, tag="wsl")
                for s in range(SC):
                    tw = sb.tile([P, 2], F32, tag="tw")
                    nc.sync.dma_start(
                        tw[:],
                        scr["tid_w"][e * CAP + s * P:e * CAP + (s + 1) * P, :])
                    tidx = sb.tile([P, 1], I32, tag="tidx")
                    nc.vector.tensor_copy(tidx[:], tw[:, 0:1])
                    nc.vector.tensor_copy(wsl[:, s:s + 1], tw[:, 1:2])
                    xg = sb.tile([P, H], BF16, tag="xg", bufs=3)
                    nc.gpsimd.indirect_dma_start(
                        out=xg[:], out_offset=None, in_=scr["h2_all"][:],
                        in_offset=bass.IndirectOffsetOnAxis(ap=tidx[:, :1],
                                                            axis=0))
                    for kc in range(HC):
                        transpose_to(
                            xT[:, kc * CAP + s * P:kc * CAP + (s + 1) * P],
                            xg[:, kc * P:(kc + 1) * P], BF16)
                if kind == 'quantum':
                    nc.vector.tensor_scalar_mul(wsl[:], wsl[:], 1.1)

                wg_sb = load_w(f"wg{e}", H, ic, tag="wgu", bufs=3)
                wu_sb = load_w(f"wu{e}", H, ic, tag="wgu", bufs=3)
                aT = sb.tile([P, mi_n * CAP], BF16, tag="aT")
                for mi in range(mi_n):
                    g_ps = ps.tile([P, CAP], F32, tag="acc")
                    u_ps = ps.tile([P, CAP], F32, tag="acc")
                    for kc in range(HC):
                        nc.tensor.matmul(
                            g_ps[:],
                            lhsT=wg_sb[:, kc * ic + mi * P:kc * ic + (mi + 1) * P],
                            rhs=xT[:, kc * CAP:(kc + 1) * CAP],
                            start=(kc == 0), stop=(kc == HC - 1))
                    for kc in range(HC):
                        nc.tensor.matmul(
                            u_ps[:],
                            lhsT=wu_sb[:, kc * ic + mi * P:kc * ic + (mi + 1) * P],
                            rhs=xT[:, kc * CAP:(kc + 1) * CAP],
                            start=(kc == 0), stop=(kc == HC - 1))
                    gs = sb.tile([P, CAP], BF16, tag="gs")
                    nc.scalar.activation(out=gs[:], in_=g_ps[:], func=AF.Silu)
                    nc.vector.tensor_tensor(
                        out=aT[:, mi * CAP:(mi + 1) * CAP],
                        in0=gs[:], in1=u_ps[:], op=ALU.mult)

                thT = None
                if kind == 'creativity':
                    wc_sb = load_w(f"wc{e}", H, P, tag="wc", bufs=2)
                    thT = sb.tile([P, SC * P], BF16, tag="thT")
                    for s in range(SC):
                        t_ps = ps.tile([P, P], F32, tag="acc")
                        for kc in range(HC):
                            nc.tensor.matmul(
                                t_ps[:],
                                lhsT=wc_sb[:, kc * P:(kc + 1) * P],
                                rhs=xT[:, kc * CAP + s * P:kc * CAP + (s + 1) * P],
                                start=(kc == 0), stop=(kc == HC - 1))
                        nc.scalar.activation(out=thT[:, s * P:(s + 1) * P],
                                             in_=t_ps[:], func=AF.Tanh)

                wd_sb = load_w(f"wd{e}", ic, H, tag="wd", bufs=2)
                for s in range(SC):
                    y_sb = sb.tile([P, H], BF16, tag="y_sb", bufs=3)
                    for i, n0 in enumerate((0, 512)):
                        y_ps = ps_y.tile([P, 512], F32, tag="ydown")
                        for mi in range(mi_n):
                            nc.tensor.matmul(
                                y_ps[:],
                                lhsT=aT[:, mi * CAP + s * P:mi * CAP + (s + 1) * P],
                                rhs=wd_sb[:, mi * H + n0:mi * H + n0 + 512],
                                start=(mi == 0),
                                stop=(mi == mi_n - 1 and kind != 'creativity'))
                        if kind == 'creativity':
                            nc.tensor.matmul(
                                y_ps[:],
                                lhsT=thT[:, s * P:(s + 1) * P],
                                rhs=cre_sel[:, n0:n0 + 512],
                                start=False, stop=True)
                        nc.vector.tensor_scalar_mul(y_sb[:, n0:n0 + 512],
                                                    y_ps[:], wsl[:, s:s + 1])
                    nc.sync.dma_start(
                        scr["ycomp"][e * CAP + s * P:e * CAP + (s + 1) * P, :],
                        y_sb[:])

            # ================= combine =================
            for tch in range(TC):
                g1 = sb.tile([P, H], BF16, tag="g1")
                nc.gpsimd.indirect_dma_start(
                    out=g1[:], out_offset=None, in_=scr["ycomp"][:],
                    in_offset=bass.IndirectOffsetOnAxis(
                        ap=slot12[:, tch * 2:tch * 2 + 1], axis=0))
                g2 = sb.tile([P, H], BF16, tag="g2")
                nc.gpsimd.indirect_dma_start(
                    out=g2[:], out_offset=None, in_=scr["ycomp"][:],
                    in_offset=bass.IndirectOffsetOnAxis(
                        ap=slot12[:, tch * 2 + 1:tch * 2 + 2], axis=0))
                pc = sb.tile([P, H], F32, tag="pc")
                nc.vector.tensor_tensor(out=pc[:], in0=g1[:], in1=g2[:],
                                        op=ALU.add)
                nc.sync.dma_start(scr["rs_in"][tch * P:(tch + 1) * P, :],
                                  pc[:])
            nc.gpsimd.collective_compute(
                "ReduceScatter", ALU.add, replica_groups=RG,
                ins=[scr["rs_in"][:]], outs=[scr["rs_out"][:]])
            rs_sb = sb.tile([P, H], F32, tag="rs_sb", bufs=1)
            nc.sync.dma_start(rs_sb[:], scr["rs_out"][:])
            out_sb = sb.tile([P, H], F32, tag="out_sb", bufs=1)
            nc.vector.tensor_tensor(out=out_sb[:], in0=x2_t[:], in1=rs_sb[:],
                                    op=ALU.add)
            nc.sync.dma_start(y_out[:], out_sb[:])


# ---------------------------------------------------------------------------
# host wrapper
# ---------------------------------------------------------------------------

_CACHE = {}


def _prep_inmaps(hidden_states, params):
    x = np.asarray(hidden_states, np.float32).reshape(T, H)
    p = params

    def np32(a):
        return np.ascontiguousarray(np.asarray(a, np.float32))

    def bf(a):
        return np.ascontiguousarray(np.asarray(a).astype(ml_dtypes.bfloat16))

    pos = np.arange(T, dtype=np.float32)
    freqs = 1.0 / (10000.0 ** (np.arange(0, HD, 2, dtype=np.float32) / HD))
    ang = pos[:, None] * freqs[None, :]
    cos_full = np.cos(ang).astype(np.float32)
    sin_full = np.sin(ang).astype(np.float32)

    qw_t = bf(np32(p['q_w']).T)
    kw_t = bf(np32(p['k_w']).T)
    vw_t = bf(np32(p['v_w']).T)
    ow_t = bf(np32(p['o_w']).T)
    rw_t = np.ascontiguousarray(np32(p['router_w']).T)

    in_maps = []
    for c in range(NCORE):
        cre = np.zeros((P, H), np.float32)
        cre[np.arange(P), c * P + np.arange(P)] = 0.2
        m = {
            "x_own": np.ascontiguousarray(x[c * P:(c + 1) * P]),
            "cos_own": np.ascontiguousarray(cos_full[c * P:(c + 1) * P]),
            "sin_own": np.ascontiguousarray(sin_full[c * P:(c + 1) * P]),
            "qw_t": qw_t, "kw_t": kw_t, "vw_t": vw_t, "ow_t": ow_t,
            "rw_t": rw_t, "cre_sel": bf(cre),
        }
        for e, kind in enumerate(KINDS):
            ic = INTER[kind] // NCORE
            ex = p['experts'][e]
            gt = np32(ex['gate']).T
            ut = np32(ex['up']).T
            dt_ = np32(ex['down']).T
            m[f"wg{e}"] = bf(gt[:, c * ic:(c + 1) * ic])
            m[f"wu{e}"] = bf(ut[:, c * ic:(c + 1) * ic])
            m[f"wd{e}"] = bf(dt_[c * ic:(c + 1) * ic, :])
            if kind == 'creativity':
                ct = np32(ex['creative']).T
                m[f"wc{e}"] = bf(ct[:, c * P:(c + 1) * P])
        in_maps.append(m)
    return in_maps


def kernel(hidden_states, params, _trace=False, _reps=1):
    key = f"nc{_reps}"
    if key not in _CACHE:
        _CACHE[key] = build_nc(_reps)
    nc = _CACHE[key]
    in_maps = _prep_inmaps(hidden_states, params)
    res = run_bass_kernel_spmd(nc, in_maps, core_ids=list(range(NCORE)),
                               trace=_trace)
    _CACHE["last"] = res
    out = np.concatenate([res.results[c]["y_out"] for c in range(NCORE)],
                         axis=0)
    return out.reshape(1, T, H).astype(np.float32)
